# revision 4
# baseline (speedup 1.0000x reference)
"""BitfieldLinear (vq_codebook) Trainium2 kernel.

y = x @ W^T + bias, W decoded from a VQ codebook:
    idx = codes & 0xFF            (basis row, 256 entries)
    r   = ((codes >> 8) & 0xFFFF) / 65535
    W   = r[:,None] * basis[idx] + scales[:,None] * (resid - 128) / 127

Sharding across 8 NeuronCores: 4-way over out_features (1024 each) x
2-way over flattened tokens (4096 each). Each core decodes its W shard
on device (gather + ACT + DVE), transposes W^T and x^T via DMA xbar,
and runs bf16 matmuls with fp32 PSUM accumulation; bias is added during
PSUM evacuation. Host only slices inputs and reassembles the output.
"""

import numpy as np

import concourse.bass as bass
import concourse.mybir as mybir
import concourse.tile as tile
from concourse.bass_utils import run_bass_kernel_spmd

# problem shape (hardcoded per harness contract)
B, S, D_IN, D_OUT, BASIS = 4, 2048, 4096, 4096, 256
N_CORES = 8
O_SHARDS, N_SHARDS = 4, 2           # grid: core = oc * N_SHARDS + nb
O_SH = D_OUT // O_SHARDS            # 1024 out-features per core
N_SH = (B * S) // N_SHARDS          # 4096 token rows per core

P = 128
KC = D_IN // P                      # 32 contraction chunks
OT = O_SH // P                      # 8 o-tiles per core
NOS = O_SH // 512                   # 2 PSUM o-slices per core
NB = N_SH // P                      # 32 token blocks per core

F32 = mybir.dt.float32
BF16 = mybir.dt.bfloat16
I32 = mybir.dt.int32

_WAIT_LIMIT = 1


def _split_sync_waits(nc):
    """walrus in this container rejects instructions with more than one
    embedded sync-wait command; hoist the excess onto same-engine NoOps."""
    ctr = 0
    for f in nc.m.functions:
        for bb in f.blocks:
            new = []
            changed = False
            for inst in bb.instructions:
                si = inst.sync_info
                if si is not None and si.on_wait and len(si.on_wait) > _WAIT_LIMIT:
                    waits = list(si.on_wait)
                    excess, keep = waits[:-_WAIT_LIMIT], waits[-_WAIT_LIMIT:]
                    for i in range(0, len(excess), _WAIT_LIMIT):
                        ctr += 1
                        new.append(mybir.InstNoOp(
                            name=f"I-waitsplit-{ctr}",
                            engine=inst.engine,
                            ins=[], outs=[],
                            sync_info=mybir.SyncInfo(
                                on_wait=excess[i:i + _WAIT_LIMIT], on_update=[]),
                        ))
                    si.on_wait = keep
                    changed = True
                new.append(inst)
            if changed:
                bb.instructions = new


def _build_program():
    nc = bass.Bass()
    Alu = mybir.AluOpType
    Act = mybir.ActivationFunctionType

    x_in = nc.dram_tensor("x_sh", [N_SH, D_IN], F32, kind="ExternalInput")
    codes_in = nc.dram_tensor("codes_sh", [O_SH], I32, kind="ExternalInput")
    basis_in = nc.dram_tensor("basis", [BASIS, D_IN], F32, kind="ExternalInput")
    resid_in = nc.dram_tensor("resid_sh", [O_SH, D_IN], I32, kind="ExternalInput")
    scales_in = nc.dram_tensor("scales_sh", [O_SH], F32, kind="ExternalInput")
    bias_in = nc.dram_tensor("bias_sh", [O_SH], F32, kind="ExternalInput")
    y_out = nc.dram_tensor("y_sh", [N_SH, O_SH], F32, kind="ExternalOutput")

    with tile.TileContext(nc) as tc:
        with (
            tc.tile_pool(name="const", bufs=1) as cpool,
            tc.tile_pool(name="dram", bufs=1, space="DRAM") as dpool,
            tc.tile_pool(name="resid", bufs=2) as rpool,
            tc.tile_pool(name="gath", bufs=2) as gpool,
            tc.tile_pool(name="r1", bufs=2) as r1pool,
            tc.tile_pool(name="wnat", bufs=2) as wpool,
            tc.tile_pool(name="wt", bufs=1) as wtpool,
            tc.tile_pool(name="xbf", bufs=2) as xpool,
            tc.tile_pool(name="xt", bufs=2) as xtpool,
            tc.tile_pool(name="y", bufs=2) as ypool,
            tc.tile_pool(name="psum", bufs=4, space="PSUM") as pspool,
            tc.tile_pool(name="psbias", bufs=1, space="PSUM") as pbpool,
        ):
            # ---- prep: per-partition decode scalars -------------------
            # layout [p, t]: element (p, t) holds value for o = t*128 + p
            codes_pp = cpool.tile([P, OT], I32)
            nc.sync.dma_start(codes_pp[:], codes_in.rearrange("(t p) -> p t", p=P))
            idx_pp = cpool.tile([P, OT], I32)
            nc.vector.tensor_scalar(idx_pp[:], codes_pp[:], 0xFF, None,
                                    Alu.bitwise_and)
            rq_pp = cpool.tile([P, OT], I32)
            nc.vector.tensor_scalar(rq_pp[:], codes_pp[:], 8, 0xFFFF,
                                    Alu.logical_shift_right, Alu.bitwise_and)
            r_pp = cpool.tile([P, OT], F32)
            nc.scalar.activation(r_pp[:], rq_pp[:], Act.Copy, scale=1.0 / 65535.0)
            s_pp = cpool.tile([P, OT], F32)
            nc.sync.dma_start(s_pp[:], scales_in.rearrange("(t p) -> p t", p=P))
            sv_pp = cpool.tile([P, OT], F32)
            nc.vector.tensor_scalar_mul(sv_pp[:], s_pp[:], 1.0 / 127.0)
            bv_pp = cpool.tile([P, OT], F32)
            nc.vector.tensor_scalar_mul(bv_pp[:], s_pp[:], -128.0 / 127.0)

            # ---- basis table -> bf16 DRAM scratch ---------------------
            basis_bf = dpool.tile([BASIS, D_IN], BF16)
            for h in range(BASIS // P):
                stage = gpool.tile([P, D_IN], BF16, tag="g_t", name=f"stage{h}")
                nc.gpsimd.dma_start(stage[:], basis_in[h * P:(h + 1) * P, :])
                nc.sync.dma_start(basis_bf[h * P:(h + 1) * P, :], stage[:])

            # ---- bias broadcast [128, O_SH] via ones (x) bias_row -----
            bias_row = cpool.tile([1, O_SH], F32)
            nc.sync.dma_start(bias_row[:], bias_in[None, :])
            ones_row = cpool.tile([1, P], F32)
            nc.vector.memset(ones_row[:], 1.0)
            bias_bc = cpool.tile([P, O_SH], F32)
            for os in range(NOS):
                pb = pbpool.tile([P, 512], F32)
                nc.tensor.matmul(pb[:], lhsT=ones_row[:, :],
                                 rhs=bias_row[:, os * 512:(os + 1) * 512],
                                 start=True, stop=True)
                nc.scalar.copy(bias_bc[:, os * 512:(os + 1) * 512], pb[:])

            # ---- W^T build -------------------------------------------
            # wt[os] layout [128 i_lo, 4 j, 32 k, 128 o']:
            #   element (p, j, k, o') = W^T[i = k*128+p, o = os*512 + j*128 + o']
            wts = [wtpool.tile([P, 4, KC, P], BF16, tag=f"wt{os}", name=f"wt{os}")
                   for os in range(NOS)]
            for os in range(NOS):
                for j in range(4):
                    t = os * 4 + j
                    resid_t = rpool.tile([P, D_IN], I32)
                    nc.sync.dma_start(resid_t[:], resid_in[t * P:(t + 1) * P, :])
                    g_t = gpool.tile([P, D_IN], BF16, tag="g_t")
                    nc.gpsimd.indirect_dma_start(
                        out=g_t[:], out_offset=None, in_=basis_bf[:],
                        in_offset=bass.IndirectOffsetOnAxis(
                            ap=idx_pp[:, t:t + 1], axis=0))
                    # r1 = scales/127 * q - 128*scales/127  (per-partition APs)
                    r1_t = r1pool.tile([P, D_IN], BF16)
                    nc.scalar.activation(r1_t[:], resid_t[:], Act.Identity,
                                         bias=bv_pp[:, t:t + 1],
                                         scale=sv_pp[:, t:t + 1])
                    # w = g * r + r1
                    w_t = wpool.tile([P, D_IN], BF16)
                    nc.vector.scalar_tensor_tensor(
                        w_t[:], g_t[:], r_pp[:, t:t + 1], r1_t[:],
                        op0=Alu.mult, op1=Alu.add)
                    nc.sync.dma_start_transpose(wts[os][:, j, :, :], w_t[:])

            # ---- main loop: stream x blocks, matmul, evac -------------
            for nb in range(NB):
                x_bf = xpool.tile([P, D_IN], BF16)
                nc.gpsimd.dma_start(x_bf[:], x_in[nb * P:(nb + 1) * P, :])
                xT = xtpool.tile([P, KC, P], BF16)
                nc.sync.dma_start_transpose(xT[:], x_bf[:])

                ps = [pspool.tile([P, 512], F32, tag="mm", name=f"psmm{nb}_{os}")
                      for os in range(NOS)]
                for k in range(KC):
                    for os in range(NOS):
                        nc.tensor.matmul(ps[os][:], lhsT=xT[:, k, :],
                                         rhs=wts[os][:, :, k, :],
                                         start=(k == 0), stop=(k == KC - 1))
                y_sb = ypool.tile([P, O_SH], F32)
                for os in range(NOS):
                    nc.vector.tensor_add(
                        y_sb[:, os * 512:(os + 1) * 512], ps[os][:],
                        bias_bc[:, os * 512:(os + 1) * 512])
                nc.sync.dma_start(y_out[nb * P:(nb + 1) * P, :], y_sb[:])

    _split_sync_waits(nc)
    return nc


_program_cache = {}


def _get_program():
    if "nc" not in _program_cache:
        _program_cache["nc"] = _build_program()
    return _program_cache["nc"]


def kernel(x, codes, basis_table, residual_q, residual_scales, bias):
    x = np.ascontiguousarray(np.asarray(x, dtype=np.float32))
    codes = np.ascontiguousarray(np.asarray(codes, dtype=np.int32))
    basis_table = np.ascontiguousarray(np.asarray(basis_table, dtype=np.float32))
    residual_q = np.ascontiguousarray(np.asarray(residual_q, dtype=np.int32))
    residual_scales = np.ascontiguousarray(
        np.asarray(residual_scales, dtype=np.float32))
    bias = np.ascontiguousarray(np.asarray(bias, dtype=np.float32))

    x2 = x.reshape(B * S, D_IN)
    in_maps = []
    for core in range(N_CORES):
        oc, nb = divmod(core, N_SHARDS)
        osl = slice(oc * O_SH, (oc + 1) * O_SH)
        nsl = slice(nb * N_SH, (nb + 1) * N_SH)
        in_maps.append({
            "x_sh": np.ascontiguousarray(x2[nsl]),
            "codes_sh": np.ascontiguousarray(codes[osl]),
            "basis": basis_table,
            "resid_sh": np.ascontiguousarray(residual_q[osl]),
            "scales_sh": np.ascontiguousarray(residual_scales[osl]),
            "bias_sh": np.ascontiguousarray(bias[osl]),
        })

    nc = _get_program()
    res = run_bass_kernel_spmd(nc, in_maps, core_ids=list(range(N_CORES)))

    y = np.empty((B * S, D_OUT), dtype=np.float32)
    for core in range(N_CORES):
        oc, nb = divmod(core, N_SHARDS)
        y[nb * N_SH:(nb + 1) * N_SH, oc * O_SH:(oc + 1) * O_SH] = \
            res.results[core]["y_sh"]
    return y.reshape(B, S, D_OUT)


# revision 5
# speedup vs baseline: 1.0513x; 1.0513x over previous
"""BitfieldLinear (vq_codebook) Trainium2 kernel.

y = x @ W^T + bias, W decoded from a VQ codebook:
    idx = codes & 0xFF            (basis row, 256 entries)
    r   = ((codes >> 8) & 0xFFFF) / 65535
    W   = r[:,None] * basis[idx] + scales[:,None] * (resid - 128) / 127

Sharding across 8 NeuronCores: 4-way over out_features (1024 each) x
2-way over flattened tokens (4096 each). Each core decodes its W shard
on device (gather + ACT + DVE), stages W to DRAM and transposes W^T /
x^T via DMA xbar, then runs bf16 matmuls with fp32 PSUM accumulation;
bias is added during PSUM evacuation. Host only slices inputs and
reassembles the output.
"""

import numpy as np

import concourse.bass as bass
import concourse.mybir as mybir
import concourse.tile as tile
from concourse.bass_utils import run_bass_kernel_spmd

# problem shape (hardcoded per harness contract)
B, S, D_IN, D_OUT, BASIS = 4, 2048, 4096, 4096, 256
N_CORES = 8
O_SHARDS, N_SHARDS = 4, 2           # grid: core = oc * N_SHARDS + nb
O_SH = D_OUT // O_SHARDS            # 1024 out-features per core
N_SH = (B * S) // N_SHARDS          # 4096 token rows per core

P = 128
KC = D_IN // P                      # 32 contraction chunks
OT = O_SH // P                      # 8 o-tiles per core
NOS = O_SH // 512                   # 2 PSUM o-slices per core
NB = N_SH // P                      # 32 token blocks per core
HALF = D_IN // 2

F32 = mybir.dt.float32
BF16 = mybir.dt.bfloat16
I32 = mybir.dt.int32

_WAIT_LIMIT = 1


def _split_sync_waits(nc):
    """walrus in this container rejects instructions with more than one
    embedded sync-wait command; hoist the excess onto same-engine NoOps."""
    ctr = 0
    for f in nc.m.functions:
        for bb in f.blocks:
            new = []
            changed = False
            for inst in bb.instructions:
                si = inst.sync_info
                if si is not None and si.on_wait and len(si.on_wait) > _WAIT_LIMIT:
                    waits = list(si.on_wait)
                    excess, keep = waits[:-_WAIT_LIMIT], waits[-_WAIT_LIMIT:]
                    for i in range(0, len(excess), _WAIT_LIMIT):
                        ctr += 1
                        new.append(mybir.InstNoOp(
                            name=f"I-waitsplit-{ctr}",
                            engine=inst.engine,
                            ins=[], outs=[],
                            sync_info=mybir.SyncInfo(
                                on_wait=excess[i:i + _WAIT_LIMIT], on_update=[]),
                        ))
                    si.on_wait = keep
                    changed = True
                new.append(inst)
            if changed:
                bb.instructions = new


def _build_program():
    nc = bass.Bass()
    Alu = mybir.AluOpType
    Act = mybir.ActivationFunctionType

    x_in = nc.dram_tensor("x_sh", [N_SH, D_IN], F32, kind="ExternalInput")
    codes_in = nc.dram_tensor("codes_sh", [O_SH], I32, kind="ExternalInput")
    basis_in = nc.dram_tensor("basis", [BASIS, D_IN], F32, kind="ExternalInput")
    resid_in = nc.dram_tensor("resid_sh", [O_SH, D_IN], I32, kind="ExternalInput")
    scales_in = nc.dram_tensor("scales_sh", [O_SH], F32, kind="ExternalInput")
    bias_in = nc.dram_tensor("bias_sh", [O_SH], F32, kind="ExternalInput")
    y_out = nc.dram_tensor("y_sh", [N_SH, O_SH], F32, kind="ExternalOutput")

    with tile.TileContext(nc) as tc:
        with (
            tc.tile_pool(name="const", bufs=1) as cpool,
            tc.tile_pool(name="dram", bufs=1, space="DRAM") as dpool,
            tc.tile_pool(name="resid", bufs=2) as rpool,
            tc.tile_pool(name="gath", bufs=2) as gpool,
            tc.tile_pool(name="r1", bufs=2) as r1pool,
            tc.tile_pool(name="wnat", bufs=2) as wpool,
            tc.tile_pool(name="wt", bufs=1) as wtpool,
            tc.tile_pool(name="xbf", bufs=3) as xpool,
            tc.tile_pool(name="xt", bufs=3) as xtpool,
            tc.tile_pool(name="y", bufs=2) as ypool,
            tc.tile_pool(name="psum", bufs=4, space="PSUM") as pspool,
            tc.tile_pool(name="psbias", bufs=1, space="PSUM") as pbpool,
        ):
            # ---- prep: per-partition decode scalars -------------------
            # layout [p, t]: element (p, t) holds value for o = t*128 + p
            codes_pp = cpool.tile([P, OT], I32)
            nc.sync.dma_start(codes_pp[:], codes_in.rearrange("(t p) -> p t", p=P))
            idx_pp = cpool.tile([P, OT], I32)
            nc.vector.tensor_scalar(idx_pp[:], codes_pp[:], 0xFF, None,
                                    Alu.bitwise_and)
            rq_pp = cpool.tile([P, OT], I32)
            nc.vector.tensor_scalar(rq_pp[:], codes_pp[:], 8, 0xFFFF,
                                    Alu.logical_shift_right, Alu.bitwise_and)
            r_pp = cpool.tile([P, OT], F32)
            nc.scalar.activation(r_pp[:], rq_pp[:], Act.Copy, scale=1.0 / 65535.0)
            s_pp = cpool.tile([P, OT], F32)
            nc.sync.dma_start(s_pp[:], scales_in.rearrange("(t p) -> p t", p=P))
            sv_pp = cpool.tile([P, OT], F32)
            nc.vector.tensor_scalar_mul(sv_pp[:], s_pp[:], 1.0 / 127.0)
            bv_pp = cpool.tile([P, OT], F32)
            nc.vector.tensor_scalar_mul(bv_pp[:], s_pp[:], -128.0 / 127.0)

            # ---- basis table -> bf16 DRAM scratch ---------------------
            basis_bf = dpool.tile([BASIS, D_IN], BF16)
            for h in range(BASIS // P):
                stage = gpool.tile([P, D_IN], BF16, tag="g_t", name=f"stage{h}")
                nc.gpsimd.dma_start(stage[:], basis_in[h * P:(h + 1) * P, :])
                nc.sync.dma_start(basis_bf[h * P:(h + 1) * P, :], stage[:])

            # ---- bias broadcast [128, O_SH] via ones (x) bias_row -----
            bias_row = cpool.tile([1, O_SH], F32)
            nc.sync.dma_start(bias_row[:], bias_in[None, :])
            ones_row = cpool.tile([1, P], F32)
            nc.vector.memset(ones_row[:], 1.0)
            bias_bc = cpool.tile([P, O_SH], F32)
            for os in range(NOS):
                pb = pbpool.tile([P, 512], F32, name=f"pb{os}")
                nc.tensor.matmul(pb[:], lhsT=ones_row[:, :],
                                 rhs=bias_row[:, os * 512:(os + 1) * 512],
                                 start=True, stop=True)
                nc.scalar.copy(bias_bc[:, os * 512:(os + 1) * 512], pb[:])

            # ---- W decode -> DRAM staging -> W^T tiles ----------------
            # w_dram [O_SH, D_IN] bf16; wt[os] [128 i_lo, 32 k, 512 o']:
            #   wt[os][p, k, o'] = W^T[i = k*128+p, o = os*512 + o']
            w_dram = dpool.tile([O_SH, D_IN], BF16)
            wts = [wtpool.tile([P, KC, 512], BF16, tag=f"wt{os}", name=f"wt{os}")
                   for os in range(NOS)]
            for os in range(NOS):
                for j in range(4):
                    t = os * 4 + j
                    g_t = gpool.tile([P, D_IN], BF16, tag="g_t", name=f"g{t}")
                    nc.gpsimd.indirect_dma_start(
                        out=g_t[:], out_offset=None, in_=basis_bf[:],
                        in_offset=bass.IndirectOffsetOnAxis(
                            ap=idx_pp[:, t:t + 1], axis=0))
                    for hf in range(2):
                        hs = slice(hf * HALF, (hf + 1) * HALF)
                        resid_t = rpool.tile([P, HALF], I32, tag="res",
                                             name=f"res{t}_{hf}")
                        nc.sync.dma_start(resid_t[:],
                                          resid_in[t * P:(t + 1) * P, hs])
                        # r1 = scales/127 * q - 128*scales/127
                        r1_t = r1pool.tile([P, HALF], BF16, tag="r1",
                                           name=f"r1_{t}_{hf}")
                        nc.scalar.activation(r1_t[:], resid_t[:], Act.Identity,
                                             bias=bv_pp[:, t:t + 1],
                                             scale=sv_pp[:, t:t + 1])
                        # w = g * r + r1
                        w_t = wpool.tile([P, HALF], BF16, tag="w",
                                         name=f"w_{t}_{hf}")
                        nc.vector.scalar_tensor_tensor(
                            w_t[:], g_t[:, hs], r_pp[:, t:t + 1], r1_t[:],
                            op0=Alu.mult, op1=Alu.add)
                        nc.sync.dma_start(w_dram[t * P:(t + 1) * P, hs], w_t[:])
                # one big contiguous transpose per o-slice (DRAM -> SBUF)
                nc.sync.dma_start_transpose(
                    wts[os][:], w_dram[os * 512:(os + 1) * 512, :])

            # ---- main loop: stream x blocks, matmul, evac -------------
            for nb in range(NB):
                x_bf = xpool.tile([P, D_IN], BF16, tag="xbf", name=f"xbf{nb}")
                nc.gpsimd.dma_start(x_bf[:], x_in[nb * P:(nb + 1) * P, :])
                xT = xtpool.tile([P, KC, P], BF16, tag="xt", name=f"xt{nb}")
                # scalar-engine HWDGE ring: parallel to the sync ring
                nc.scalar.dma_start_transpose(xT[:], x_bf[:])

                y_sb = ypool.tile([P, O_SH], F32, tag="y", name=f"y{nb}")
                for os in range(NOS):
                    ps = pspool.tile([P, 512], F32, tag="mm",
                                     name=f"psmm{nb}_{os}")
                    for k in range(KC):
                        nc.tensor.matmul(ps[:], lhsT=xT[:, k, :],
                                         rhs=wts[os][:, k, :],
                                         start=(k == 0), stop=(k == KC - 1))
                    nc.vector.tensor_add(
                        y_sb[:, os * 512:(os + 1) * 512], ps[:],
                        bias_bc[:, os * 512:(os + 1) * 512])
                nc.sync.dma_start(y_out[nb * P:(nb + 1) * P, :], y_sb[:])

    _split_sync_waits(nc)
    return nc


_program_cache = {}


def _get_program():
    if "nc" not in _program_cache:
        _program_cache["nc"] = _build_program()
    return _program_cache["nc"]


def kernel(x, codes, basis_table, residual_q, residual_scales, bias):
    x = np.ascontiguousarray(np.asarray(x, dtype=np.float32))
    codes = np.ascontiguousarray(np.asarray(codes, dtype=np.int32))
    basis_table = np.ascontiguousarray(np.asarray(basis_table, dtype=np.float32))
    residual_q = np.ascontiguousarray(np.asarray(residual_q, dtype=np.int32))
    residual_scales = np.ascontiguousarray(
        np.asarray(residual_scales, dtype=np.float32))
    bias = np.ascontiguousarray(np.asarray(bias, dtype=np.float32))

    x2 = x.reshape(B * S, D_IN)
    in_maps = []
    for core in range(N_CORES):
        oc, nb = divmod(core, N_SHARDS)
        osl = slice(oc * O_SH, (oc + 1) * O_SH)
        nsl = slice(nb * N_SH, (nb + 1) * N_SH)
        in_maps.append({
            "x_sh": np.ascontiguousarray(x2[nsl]),
            "codes_sh": np.ascontiguousarray(codes[osl]),
            "basis": basis_table,
            "resid_sh": np.ascontiguousarray(residual_q[osl]),
            "scales_sh": np.ascontiguousarray(residual_scales[osl]),
            "bias_sh": np.ascontiguousarray(bias[osl]),
        })

    nc = _get_program()
    res = run_bass_kernel_spmd(nc, in_maps, core_ids=list(range(N_CORES)))

    y = np.empty((B * S, D_OUT), dtype=np.float32)
    for core in range(N_CORES):
        oc, nb = divmod(core, N_SHARDS)
        y[nb * N_SH:(nb + 1) * N_SH, oc * O_SH:(oc + 1) * O_SH] = \
            res.results[core]["y_sh"]
    return y.reshape(B, S, D_OUT)
